# revision 9
# baseline (speedup 1.0000x reference)
"""
DecisionTransformer GPT2 attention on 8 Trainium2 NeuronCores.

Sharding: core c <- batch b = c//4, heads [4*(c%4), 4*(c%4)+4).
  - c_attn column-sharded (each core computes Q/K/V only for its 4 heads)
  - c_proj row-sharded (each core computes a partial [S, D] output);
    the 4-way partial sums per batch (the "all-reduce") happen on host
    during unshard, together with the b_proj bias add.

Per-core device kernel (all fp32):
  xT[1024,2048] (pre-transposed on host) -> QT/KT [j,s] via PE, V [s,hd] via PE.
  Scores are computed TWICE (S[q,k] and S^T[k,q]) -- recomputing on the PE is
  far cheaper than transposing P on this hardware.
    S-path:  S[q,k] -> exp (ACT, scale=1/8, row-sum via accum_out) -> causal
             mask (fused DVE tensor_tensor_reduce on the diagonal block) ->
             normalize by 1/l -> DMA out as attn_weights.  Upper triangle is
             never written; output buffers are pre-zeroed by the runtime.
    S^T-path: S^T[k,q] -> exp -> triangular mask -> AV accumulation (PE,
             2 heads column-packed per matmul pair) -> ctx^T.
  ctx^T normalized by 1/l (broadcast from a PE-transposed reciprocal tile),
  then the output projection vs the core's 256 rows of w_proj.

No max-subtraction in softmax: scores/8 are bounded (|s|<~3) so exp is safe,
and masked entries are exactly 0 by construction (matches jnp softmax of
finfo.min-masked scores bit-for-bit at fp32 tolerance).
"""

import sys

if "/opt/trn_rl_repo" not in sys.path:
    sys.path.insert(0, "/opt/trn_rl_repo")

import numpy as np

B, S, D, H = 2, 2048, 1024, 16
HD = D // H            # 64
NCORES = 8
HPC = 4                # heads per core
JC = HPC * HD          # 256 per-core qkv column count
P = 128                # partitions
NQT = S // P           # 16 q tiles
SC = 512               # score chunk (one PSUM bank of fp32)
NSC = S // SC          # 4
DCH = D // P           # 8 contraction chunks for the qkv projection

_cache = {}


def _build_program(phases=("s", "av", "proj"), mm_dtype="float32"):
    import concourse.mybir as mybir
    import concourse.tile as tile
    from concourse import bacc

    f32 = mybir.dt.float32
    AF = mybir.ActivationFunctionType
    ALU = mybir.AluOpType
    AX = mybir.AxisListType

    nc = bacc.Bacc("TRN2", target_bir_lowering=False, debug=False,
                   num_devices=NCORES)
    mmdt = getattr(mybir.dt, mm_dtype)

    def mm(out, lhsT, rhs, **kw):
        nc.tensor.matmul(out, lhsT.bitcast(mmdt), rhs.bitcast(mmdt), **kw)

    xT = nc.dram_tensor("xT", [D, S], f32, kind="ExternalInput").ap()
    wq = nc.dram_tensor("wq", [D, JC], f32, kind="ExternalInput").ap()
    wk = nc.dram_tensor("wk", [D, JC], f32, kind="ExternalInput").ap()
    wv = nc.dram_tensor("wv", [D, JC], f32, kind="ExternalInput").ap()
    bq = nc.dram_tensor("bq", [JC, 1], f32, kind="ExternalInput").ap()
    bk = nc.dram_tensor("bk", [JC, 1], f32, kind="ExternalInput").ap()
    bv = nc.dram_tensor("bv", [1, JC], f32, kind="ExternalInput").ap()
    wp = nc.dram_tensor("wp", [JC, D], f32, kind="ExternalInput").ap()
    pw = nc.dram_tensor("pw", [HPC, S, S], f32, kind="ExternalOutput").ap()
    po = nc.dram_tensor("po", [S, D], f32, kind="ExternalOutput").ap()

    with tile.TileContext(nc) as tc:
        from contextlib import ExitStack
        with ExitStack() as ctx:
            # ---------- long-lived pools (whole kernel) --------------------
            const = ctx.enter_context(tc.tile_pool(name="const", bufs=1))
            qkp = ctx.enter_context(tc.tile_pool(name="qkp", bufs=1))
            ctxp = ctx.enter_context(tc.tile_pool(name="ctxp", bufs=1))
            rp = ctx.enter_context(tc.tile_pool(name="rp", bufs=1))
            lpp = ctx.enter_context(tc.tile_pool(name="lpp", bufs=4))

            # ---- constants ------------------------------------------------
            ones1 = const.tile([1, P], f32)
            nc.gpsimd.memset(ones1[:], 1.0)
            # trid[v]: [128, 512] causal mask for an S-path diagonal chunk,
            # mask[p, f] = 1 iff p - f + 128*v >= 0
            trid = const.tile([P, 4 * SC], f32)
            nc.gpsimd.memset(trid[:], 1.0)
            for v in range(4):
                nc.gpsimd.affine_select(
                    out=trid[:, v * SC:(v + 1) * SC],
                    in_=trid[:, v * SC:(v + 1) * SC],
                    compare_op=ALU.is_ge, fill=0.0,
                    base=P * v, channel_multiplier=1, pattern=[[-1, SC]])
            # triT: [128,128] mask[p, f] = 1 iff f - p >= 0
            triT = const.tile([P, P], f32)
            nc.gpsimd.memset(triT[:], 1.0)
            nc.gpsimd.affine_select(
                out=triT[:], in_=triT[:], compare_op=ALU.is_ge, fill=0.0,
                base=0, channel_multiplier=-1, pattern=[[1, P]])
            bq_sb = const.tile([P, 2], f32, tag="bq")
            bk_sb = const.tile([P, 2], f32, tag="bk")
            for jt in range(2):
                nc.sync.dma_start(bq_sb[:, jt:jt + 1], bq[jt * P:(jt + 1) * P, :])
                nc.sync.dma_start(bk_sb[:, jt:jt + 1], bk[jt * P:(jt + 1) * P, :])
            bv_sb = const.tile([1, JC], f32, tag="bv")
            nc.sync.dma_start(bv_sb[:], bv[:])

            # long-lived data tiles
            qt_sb = [qkp.tile([P, S], f32, tag=f"qt{j}", name=f"qt{j}")
                     for j in range(2)]
            kt_sb = [qkp.tile([P, S], f32, tag=f"kt{j}", name=f"kt{j}")
                     for j in range(2)]
            v_sb = qkp.tile([P, NQT * JC], f32, tag="v")
            ctx_sb = [ctxp.tile([P, S], f32, tag=f"ctx{j}", name=f"ctx{j}")
                      for j in range(2)]
            rst = [rp.tile([P, NQT], f32, tag=f"rst{h}", name=f"rst{h}")
                   for h in range(HPC)]

            # ---------- QKV phase (xT + qkv weights live only here) --------
            with tc.tile_pool(name="xpool", bufs=1) as xpool, \
                 tc.tile_pool(name="wqkv", bufs=1) as wqkv, \
                 tc.tile_pool(name="ps_qkv", bufs=4, space="PSUM") as ps_qkv:
                xT_sb = xpool.tile([P, DCH * S], f32)    # chunk dc at cols dc*S
                nc.sync.dma_start(xT_sb.rearrange("p (c s) -> p c s", c=DCH),
                                  xT.rearrange("(c p) s -> p c s", p=P))
                w_sb = {}
                for name, drm in (("wq", wq), ("wk", wk), ("wv", wv)):
                    t = wqkv.tile([P, DCH * JC], f32, tag=name, name=name)
                    nc.sync.dma_start(t.rearrange("p (c j) -> p c j", c=DCH),
                                      drm.rearrange("(c p) j -> p c j", p=P))
                    w_sb[name] = t

                # QT/KT: [j, s] layout; pair tile jt holds heads 2jt
                # (parts 0-63) and 2jt+1 (parts 64-127).
                for jt in range(2):
                    for name, dst, bias in (("wq", qt_sb[jt], bq_sb),
                                            ("wk", kt_sb[jt], bk_sb)):
                        for sc in range(NSC):
                            ps = ps_qkv.tile([P, SC], f32, tag="qk")
                            for dc in range(DCH):
                                mm(
                                    ps[:],
                                    w_sb[name][:, dc * JC + jt * P:
                                               dc * JC + (jt + 1) * P],
                                    xT_sb[:, dc * S + sc * SC:
                                          dc * S + (sc + 1) * SC],
                                    start=(dc == 0), stop=(dc == DCH - 1))
                            nc.scalar.activation(dst[:, sc * SC:(sc + 1) * SC],
                                                 ps[:], AF.Identity,
                                                 bias=bias[:, jt:jt + 1])
                # V: [s, hd] layout, s-tile st at cols st*JC of v_sb
                for st in range(NQT):
                    psv = ps_qkv.tile([P, JC], f32, tag="qk")
                    for dc in range(DCH):
                        mm(
                            psv[:],
                            xT_sb[:, dc * S + st * P:dc * S + (st + 1) * P],
                            w_sb["wv"][:, dc * JC:(dc + 1) * JC],
                            start=(dc == 0), stop=False)
                    nc.tensor.matmul(psv[:], ones1[:1, :P], bv_sb[:1, :],
                                     start=False, stop=True)
                    nc.vector.tensor_copy(v_sb[:, st * JC:(st + 1) * JC],
                                          psv[:])

            # ---------- attention (pbuf/ptb reuse the freed xT space) ------
            with tc.tile_pool(name="pbufp", bufs=3) as pbufp, \
                 tc.tile_pool(name="ptbp", bufs=3) as ptbp:
                for jt in range(2 if ("s" in phases or "av" in phases) else 0):
                    # ---- S path: P tiles + row sums ----
                    with tc.tile_pool(name=f"ps_s{jt}", bufs=2 if "s" in phases else 1,
                                      space="PSUM") as ps_s:
                        for t in range(NQT if "s" in phases else 0):
                            nfull = t // 4
                            off = P * (t % 4)
                            ncols = P * (t + 1)
                            fullw = nfull * SC
                            diagw = off + P
                            for l2 in range(2):
                                hl = 2 * jt + l2
                                q_lhsT = qt_sb[jt][64 * l2:64 * (l2 + 1),
                                                   t * P:(t + 1) * P]
                                ps = ps_s.tile([P, 2048], f32, tag="s")
                                for c in range(nfull):
                                    mm(
                                        ps[:, c * SC:(c + 1) * SC], q_lhsT,
                                        kt_sb[jt][64 * l2:64 * (l2 + 1),
                                                  c * SC:(c + 1) * SC],
                                        start=True, stop=True,
                                        tile_position=(64 * l2, 0))
                                mm(
                                    ps[:, fullw:fullw + diagw], q_lhsT,
                                    kt_sb[jt][64 * l2:64 * (l2 + 1),
                                              fullw:fullw + diagw],
                                    start=True, stop=True,
                                    tile_position=(64 * l2, 0))
                                pb = pbufp.tile([P, S], f32, tag="pb")
                                lp = lpp.tile([P, 4], f32, tag="lp")
                                if nfull > 0:
                                    nc.scalar.activation(
                                        pb[:, :fullw], ps[:, :fullw],
                                        AF.Exp, scale=0.125,
                                        accum_out=lp[:, 0:1])
                                nc.scalar.activation(pb[:, fullw:ncols],
                                                     ps[:, fullw:ncols],
                                                     AF.Exp, scale=0.125)
                                # causal mask on the diag chunk, then its row sum
                                # (tensor_tensor_reduce crashes TRN2 here)
                                nc.vector.tensor_mul(
                                    pb[:, fullw:ncols], pb[:, fullw:ncols],
                                    trid[:, (t % 4) * SC:(t % 4) * SC + diagw])
                                nc.vector.reduce_sum(lp[:, 1:2],
                                                     pb[:, fullw:ncols], AX.X)
                                rcol = rst[hl][:, t:t + 1]
                                if nfull > 0:
                                    nc.vector.reduce_sum(lp[:, 2:3],
                                                         lp[:, 0:2], AX.X)
                                    nc.vector.reciprocal(rcol, lp[:, 2:3])
                                else:
                                    nc.vector.reciprocal(rcol, lp[:, 1:2])
                                nc.vector.tensor_scalar_mul(pb[:, :ncols],
                                                            pb[:, :ncols],
                                                            rcol)
                                nc.sync.dma_start(
                                    pw[hl, t * P:(t + 1) * P, 0:ncols],
                                    pb[:, :ncols])
                    # ---- S^T path + AV ----
                    with tc.tile_pool(name=f"ps_st{jt}", bufs=3,
                                      space="PSUM") as ps_st, \
                         tc.tile_pool(name=f"ps_av{jt}", bufs=2,
                                      space="PSUM") as ps_av:
                        for qc in range(NSC if "av" in phases else 0):
                            av = ps_av.tile([P, SC], f32, tag="av")
                            kmax = 4 * qc + 4
                            for kt0 in range(0, kmax, 2):
                                for l2 in range(2):
                                    hl = 2 * jt + l2
                                    ps2 = ps_st.tile([P, 2 * SC], f32,
                                                     tag="st")
                                    for ki in range(2):
                                        kt = kt0 + ki
                                        mm(
                                            ps2[:, ki * SC:(ki + 1) * SC],
                                            kt_sb[jt][64 * l2:64 * (l2 + 1),
                                                      kt * P:(kt + 1) * P],
                                            qt_sb[jt][64 * l2:64 * (l2 + 1),
                                                      qc * SC:(qc + 1) * SC],
                                            start=True, stop=True,
                                            tile_position=(64 * l2, 0))
                                    ptb = ptbp.tile([P, 2 * SC], f32,
                                                    tag="ptb")
                                    nc.scalar.activation(ptb[:], ps2[:],
                                                         AF.Exp, scale=0.125)
                                    for ki in range(2):
                                        kt = kt0 + ki
                                        if kt // 4 == qc:   # diagonal k tile
                                            off = P * (kt % 4)
                                            nc.vector.tensor_mul(
                                                ptb[:, ki * SC + off:
                                                    ki * SC + off + P],
                                                ptb[:, ki * SC + off:
                                                    ki * SC + off + P],
                                                triT[:])
                                    for ki in range(2):
                                        kt = kt0 + ki
                                        avc0 = (P * (kt % 4)
                                                if kt // 4 == qc else 0)
                                        mm(
                                            av[64 * l2:64 * (l2 + 1),
                                               avc0:SC],
                                            v_sb[:, kt * JC + 64 * hl:
                                                 kt * JC + 64 * (hl + 1)],
                                            ptb[:, ki * SC + avc0:
                                                (ki + 1) * SC],
                                            start=(kt == 0),
                                            stop=(kt == kmax - 1),
                                            tile_position=(0, 64 * l2),
                                            skip_group_check=True)
                            nc.vector.tensor_copy(
                                ctx_sb[jt][:, qc * SC:(qc + 1) * SC], av[:])

            # ---------- output projection ----------------------------------
            with tc.tile_pool(name="wppool", bufs=1) as wppool, \
                 tc.tile_pool(name="postg", bufs=3) as postg, \
                 tc.tile_pool(name="ps_po", bufs=2, space="PSUM") as ps_po:
                wp_sb = wppool.tile([P, 2 * D], f32, tag="wp")
                nc.sync.dma_start(wp_sb.rearrange("p (c e) -> p c e", c=2),
                                  wp.rearrange("(c p) e -> p c e", p=P))
                # per-head K=64 matmuls (row-packed pairs run concurrently);
                # the 1/l normalization rides the PSUM->SBUF combine as a
                # per-partition (= per query row) scalar.
                for st in range(NQT if "proj" in phases else 0):
                    postage = postg.tile([P, D], f32, tag="po")
                    for ec in range(2):
                        pp = [ps_po.tile([P, SC], f32, tag=f"pp{h}",
                                         name=f"pp{h}") for h in range(HPC)]
                        for jt in range(2):
                            for l2 in range(2):
                                hl = 2 * jt + l2
                                mm(
                                    pp[hl][:],
                                    ctx_sb[jt][64 * l2:64 * (l2 + 1),
                                               st * P:(st + 1) * P],
                                    wp_sb[64 * l2:64 * (l2 + 1),
                                          jt * D + ec * SC:
                                          jt * D + (ec + 1) * SC],
                                    start=True, stop=True,
                                    tile_position=(64 * l2, 0))
                        sg = postage[:, ec * SC:(ec + 1) * SC]
                        nc.vector.tensor_scalar_mul(sg, pp[0][:],
                                                    rst[0][:, st:st + 1])
                        for hl in range(1, HPC):
                            nc.vector.scalar_tensor_tensor(
                                sg, pp[hl][:], rst[hl][:, st:st + 1], sg,
                                op0=ALU.mult, op1=ALU.add)
                    nc.sync.dma_start(po[st * P:(st + 1) * P, :], postage[:])

    nc.compile()
    return nc


def _get_program():
    if "nc" not in _cache:
        import os
        _cache["nc"] = _build_program(
            mm_dtype=os.environ.get("KERNEL_MM_DTYPE", "float32"))
    return _cache["nc"]


def _make_in_maps(hidden_states, w_attn, b_attn):
    hs = np.ascontiguousarray(np.asarray(hidden_states, dtype=np.float32))
    wa = np.asarray(w_attn, dtype=np.float32)
    ba = np.asarray(b_attn, dtype=np.float32)
    in_maps = []
    for c in range(NCORES):
        b = c // 4
        j0 = (c % 4) * HPC * HD
        in_maps.append({
            "xT": np.ascontiguousarray(hs[b].T),
            "wq": np.ascontiguousarray(wa[:, j0:j0 + JC]),
            "wk": np.ascontiguousarray(wa[:, D + j0:D + j0 + JC]),
            "wv": np.ascontiguousarray(wa[:, 2 * D + j0:2 * D + j0 + JC]),
            "bq": np.ascontiguousarray(ba[j0:j0 + JC].reshape(JC, 1)),
            "bk": np.ascontiguousarray(ba[D + j0:D + j0 + JC].reshape(JC, 1)),
            "bv": np.ascontiguousarray(
                ba[2 * D + j0:2 * D + j0 + JC].reshape(1, JC)),
        })
    return in_maps


def _make_wp_maps(w_proj):
    wpf = np.asarray(w_proj, dtype=np.float32)
    return [np.ascontiguousarray(wpf[(c % 4) * JC:(c % 4) * JC + JC, :])
            for c in range(NCORES)]


def kernel(hidden_states, w_attn, b_attn, w_proj, b_proj, _trace=False):
    from concourse.bass_utils import run_bass_kernel_spmd

    nc = _get_program()
    in_maps = _make_in_maps(hidden_states, w_attn, b_attn)
    wp_maps = _make_wp_maps(w_proj)
    for c in range(NCORES):
        in_maps[c]["wp"] = wp_maps[c]

    kw = {}
    if _trace:
        kw = dict(trace=True)
    res = run_bass_kernel_spmd(nc, in_maps, core_ids=list(range(NCORES)), **kw)

    attn_w = np.empty((B, H, S, S), np.float32)
    out = np.zeros((B, S, D), np.float32)
    for c in range(NCORES):
        b = c // 4
        h0 = (c % 4) * HPC
        attn_w[b, h0:h0 + HPC] = res.results[c]["pw"]
        out[b] += res.results[c]["po"]
    out += np.asarray(b_proj, dtype=np.float32)[None, None, :]
    if _trace:
        return (out, attn_w), res
    return out, attn_w


# revision 15
# speedup vs baseline: 1.0157x; 1.0157x over previous
"""
DecisionTransformer GPT2 attention on 8 Trainium2 NeuronCores.

Sharding: core c <- batch b = c//4, heads [4*(c%4), 4*(c%4)+4).
  - c_attn column-sharded (each core computes Q/K/V only for its 4 heads)
  - c_proj row-sharded (each core computes a partial [S, D] output);
    the 4-way partial sums per batch (the "all-reduce") happen on host
    during unshard, together with the b_proj bias add.

Per-core device kernel (all fp32):
  xT[1024,2048] (pre-transposed on host) -> QT/KT [j,s] via PE, V [s,hd] via PE.
  Scores are computed TWICE (S[q,k] and S^T[k,q]) -- recomputing on the PE is
  far cheaper than transposing P on this hardware.
    S-path:  S[q,k] -> exp (ACT, scale=1/8, row-sum via accum_out) -> causal
             mask (DVE tensor_mul with a precomputed mask on the diagonal
             chunk; tensor_tensor_reduce is avoided -- it crashes TRN2) ->
             normalize by 1/l -> DMA out as attn_weights.  Upper triangle is
             never written; output buffers are pre-zeroed by the runtime.
    S^T-path: S^T[k,q] -> exp -> triangular mask -> AV accumulation (PE,
             2 heads column-packed per matmul pair) -> ctx^T.
  The output projection runs per head (K=64, row-packed concurrent pairs)
  into 4 PSUM banks; the 1/l softmax normalization rides the PSUM->SBUF
  combine as a per-partition (= per query row) scalar_tensor_tensor.

No max-subtraction in softmax: scores/8 are bounded (|s|<~3) so exp is safe,
and masked entries are exactly 0 by construction (matches jnp softmax of
finfo.min-masked scores bit-for-bit at fp32 tolerance).
"""

import sys

if "/opt/trn_rl_repo" not in sys.path:
    sys.path.insert(0, "/opt/trn_rl_repo")

import numpy as np

B, S, D, H = 2, 2048, 1024, 16
HD = D // H            # 64
NCORES = 8
HPC = 4                # heads per core
JC = HPC * HD          # 256 per-core qkv column count
P = 128                # partitions
NQT = S // P           # 16 q tiles
SC = 512               # score chunk (one PSUM bank of fp32)
NSC = S // SC          # 4
DCH = D // P           # 8 contraction chunks for the qkv projection

_cache = {}


def _build_program(phases=("s", "av", "proj"), mm_dtype="float32"):
    import concourse.mybir as mybir
    import concourse.tile as tile
    from concourse import bacc

    f32 = mybir.dt.float32
    AF = mybir.ActivationFunctionType
    ALU = mybir.AluOpType
    AX = mybir.AxisListType

    nc = bacc.Bacc("TRN2", target_bir_lowering=False, debug=False,
                   num_devices=NCORES)
    mmdt = getattr(mybir.dt, mm_dtype)

    def mm(out, lhsT, rhs, **kw):
        nc.tensor.matmul(out, lhsT.bitcast(mmdt), rhs.bitcast(mmdt), **kw)

    xT = nc.dram_tensor("xT", [D, S], f32, kind="ExternalInput").ap()
    wq = nc.dram_tensor("wq", [D, JC], f32, kind="ExternalInput").ap()
    wk = nc.dram_tensor("wk", [D, JC], f32, kind="ExternalInput").ap()
    wv = nc.dram_tensor("wv", [D, JC], f32, kind="ExternalInput").ap()
    bq = nc.dram_tensor("bq", [JC, 1], f32, kind="ExternalInput").ap()
    bk = nc.dram_tensor("bk", [JC, 1], f32, kind="ExternalInput").ap()
    bv = nc.dram_tensor("bv", [1, JC], f32, kind="ExternalInput").ap()
    wp = nc.dram_tensor("wp", [JC, D], f32, kind="ExternalInput").ap()
    pw = nc.dram_tensor("pw", [HPC, S, S], f32, kind="ExternalOutput").ap()
    po = nc.dram_tensor("po", [S, D], f32, kind="ExternalOutput").ap()

    with tile.TileContext(nc) as tc:
        from contextlib import ExitStack
        with ExitStack() as ctx:
            # ---------- long-lived pools (whole kernel) --------------------
            const = ctx.enter_context(tc.tile_pool(name="const", bufs=1))
            qkp = ctx.enter_context(tc.tile_pool(name="qkp", bufs=1))
            ctxp = ctx.enter_context(tc.tile_pool(name="ctxp", bufs=1))
            rp = ctx.enter_context(tc.tile_pool(name="rp", bufs=1))
            lpp = ctx.enter_context(tc.tile_pool(name="lpp", bufs=4))

            # ---- constants ------------------------------------------------
            ones1 = const.tile([1, P], f32)
            nc.gpsimd.memset(ones1[:], 1.0)
            from concourse.masks import make_identity
            ident = const.tile([P, P], f32, tag="ident")
            make_identity(nc, ident)
            # negu: [128,128] additive causal mask for the triangular block
            # (last 128 cols of each S-path diagonal chunk): -BIG where
            # f > p (masked), else 0.  Added to the scores in PSUM via an
            # identity-lhsT matmul BEFORE exp, so exp gives exactly 0 and
            # accum_out returns the full row sum.
            NEGBIG = -3.0e38
            negu = const.tile([P, P], f32, tag="negu")
            nc.gpsimd.memset(negu[:], 0.0)
            # keep 0 where p - f >= 0 (valid), fill -BIG where masked
            # (walrus has no is_lt affine_select codegen; is_ge works)
            nc.gpsimd.affine_select(
                out=negu[:], in_=negu[:], compare_op=ALU.is_ge, fill=NEGBIG,
                base=0, channel_multiplier=1, pattern=[[-1, P]])
            # triT: [128,128] mask[p, f] = 1 iff f - p >= 0
            triT = const.tile([P, P], f32)
            nc.gpsimd.memset(triT[:], 1.0)
            nc.gpsimd.affine_select(
                out=triT[:], in_=triT[:], compare_op=ALU.is_ge, fill=0.0,
                base=0, channel_multiplier=-1, pattern=[[1, P]])
            bq_sb = const.tile([P, 2], f32, tag="bq")
            bk_sb = const.tile([P, 2], f32, tag="bk")
            for jt in range(2):
                nc.sync.dma_start(bq_sb[:, jt:jt + 1], bq[jt * P:(jt + 1) * P, :])
                nc.sync.dma_start(bk_sb[:, jt:jt + 1], bk[jt * P:(jt + 1) * P, :])
            bv_sb = const.tile([1, JC], f32, tag="bv")
            nc.sync.dma_start(bv_sb[:], bv[:])

            # long-lived data tiles
            qt_sb = [qkp.tile([P, S], f32, tag=f"qt{j}", name=f"qt{j}")
                     for j in range(2)]
            kt_sb = [qkp.tile([P, S], f32, tag=f"kt{j}", name=f"kt{j}")
                     for j in range(2)]
            v_sb = qkp.tile([P, NQT * JC], f32, tag="v")
            ctx_sb = [ctxp.tile([P, S], f32, tag=f"ctx{j}", name=f"ctx{j}")
                      for j in range(2)]
            rst = [rp.tile([P, NQT], f32, tag=f"rst{h}", name=f"rst{h}")
                   for h in range(HPC)]

            # ---------- QKV phase (xT + qkv weights live only here) --------
            with tc.tile_pool(name="xpool", bufs=1) as xpool, \
                 tc.tile_pool(name="wqkv", bufs=1) as wqkv, \
                 tc.tile_pool(name="ps_qkv", bufs=4, space="PSUM") as ps_qkv:
                xT_sb = xpool.tile([P, DCH * S], f32)    # chunk dc at cols dc*S
                nc.sync.dma_start(xT_sb.rearrange("p (c s) -> p c s", c=DCH),
                                  xT.rearrange("(c p) s -> p c s", p=P))
                w_sb = {}
                for name, drm in (("wq", wq), ("wk", wk), ("wv", wv)):
                    t = wqkv.tile([P, DCH * JC], f32, tag=name, name=name)
                    nc.sync.dma_start(t.rearrange("p (c j) -> p c j", c=DCH),
                                      drm.rearrange("(c p) j -> p c j", p=P))
                    w_sb[name] = t

                # QT/KT: [j, s] layout; pair tile jt holds heads 2jt
                # (parts 0-63) and 2jt+1 (parts 64-127).
                for jt in range(2):
                    for name, dst, bias in (("wq", qt_sb[jt], bq_sb),
                                            ("wk", kt_sb[jt], bk_sb)):
                        for sc in range(NSC):
                            ps = ps_qkv.tile([P, SC], f32, tag="qk")
                            for dc in range(DCH):
                                mm(
                                    ps[:],
                                    w_sb[name][:, dc * JC + jt * P:
                                               dc * JC + (jt + 1) * P],
                                    xT_sb[:, dc * S + sc * SC:
                                          dc * S + (sc + 1) * SC],
                                    start=(dc == 0), stop=(dc == DCH - 1))
                            nc.scalar.activation(dst[:, sc * SC:(sc + 1) * SC],
                                                 ps[:], AF.Identity,
                                                 bias=bias[:, jt:jt + 1])
                # V: [s, hd] layout, s-tile st at cols st*JC of v_sb
                for st in range(NQT):
                    psv = ps_qkv.tile([P, JC], f32, tag="qk")
                    for dc in range(DCH):
                        mm(
                            psv[:],
                            xT_sb[:, dc * S + st * P:dc * S + (st + 1) * P],
                            w_sb["wv"][:, dc * JC:(dc + 1) * JC],
                            start=(dc == 0), stop=False)
                    nc.tensor.matmul(psv[:], ones1[:1, :P], bv_sb[:1, :],
                                     start=False, stop=True)
                    nc.vector.tensor_copy(v_sb[:, st * JC:(st + 1) * JC],
                                          psv[:])

            # ---------- attention (pbuf/ptb reuse the freed xT space) ------
            with tc.tile_pool(name="pbufp", bufs=3) as pbufp, \
                 tc.tile_pool(name="ptbp", bufs=3) as ptbp:
                for jt in range(2 if ("s" in phases or "av" in phases) else 0):
                    # ---- S path: P tiles + row sums ----
                    with tc.tile_pool(name=f"ps_s{jt}", bufs=2 if "s" in phases else 1,
                                      space="PSUM") as ps_s:
                        for t in range(NQT if "s" in phases else 0):
                            nfull = t // 4
                            off = P * (t % 4)
                            ncols = P * (t + 1)
                            fullw = nfull * SC
                            diagw = off + P
                            for l2 in range(2):
                                hl = 2 * jt + l2
                                q_lhsT = qt_sb[jt][64 * l2:64 * (l2 + 1),
                                                   t * P:(t + 1) * P]
                                ps = ps_s.tile([P, 2048], f32, tag="s")
                                for c in range(nfull):
                                    mm(
                                        ps[:, c * SC:(c + 1) * SC], q_lhsT,
                                        kt_sb[jt][64 * l2:64 * (l2 + 1),
                                                  c * SC:(c + 1) * SC],
                                        start=True, stop=True,
                                        tile_position=(64 * l2, 0))
                                mm(
                                    ps[:, fullw:fullw + diagw], q_lhsT,
                                    kt_sb[jt][64 * l2:64 * (l2 + 1),
                                              fullw:fullw + diagw],
                                    start=True, stop=True,
                                    tile_position=(64 * l2, 0))
                                mm(
                                    ps[:, ncols - P:ncols], ident[:],
                                    negu[:],
                                    start=False, stop=True,
                                    skip_group_check=True)
                                pb = pbufp.tile([P, S], f32, tag="pb")
                                lp = lpp.tile([P, 4], f32, tag="lp")
                                nc.scalar.activation(
                                    pb[:, :ncols], ps[:, :ncols],
                                    AF.Exp, scale=0.125,
                                    accum_out=lp[:, 0:1])
                                rcol = rst[hl][:, t:t + 1]
                                nc.vector.reciprocal(rcol, lp[:, 0:1])
                                nc.vector.tensor_scalar_mul(pb[:, :ncols],
                                                            pb[:, :ncols],
                                                            rcol)
                                # alternate HWDGE (SP ring) / SWDGE (gpsimd)
                                # so the big P writes use two DMA streams
                                dma_eng = nc.sync if t % 2 == 0 else nc.gpsimd
                                dma_eng.dma_start(
                                    pw[hl, t * P:(t + 1) * P, 0:ncols],
                                    pb[:, :ncols])
                    # ---- S^T path + AV ----
                    with tc.tile_pool(name=f"ps_st{jt}", bufs=3,
                                      space="PSUM") as ps_st, \
                         tc.tile_pool(name=f"ps_av{jt}", bufs=2,
                                      space="PSUM") as ps_av:
                        for qc in range(NSC if "av" in phases else 0):
                            av = ps_av.tile([P, SC], f32, tag="av")
                            kmax = 4 * qc + 4
                            for kt0 in range(0, kmax, 2):
                                for l2 in range(2):
                                    hl = 2 * jt + l2
                                    ps2 = ps_st.tile([P, 2 * SC], f32,
                                                     tag="st")
                                    for ki in range(2):
                                        kt = kt0 + ki
                                        mm(
                                            ps2[:, ki * SC:(ki + 1) * SC],
                                            kt_sb[jt][64 * l2:64 * (l2 + 1),
                                                      kt * P:(kt + 1) * P],
                                            qt_sb[jt][64 * l2:64 * (l2 + 1),
                                                      qc * SC:(qc + 1) * SC],
                                            start=True, stop=True,
                                            tile_position=(64 * l2, 0))
                                    ptb = ptbp.tile([P, 2 * SC], f32,
                                                    tag="ptb")
                                    nc.scalar.activation(ptb[:], ps2[:],
                                                         AF.Exp, scale=0.125)
                                    for ki in range(2):
                                        kt = kt0 + ki
                                        if kt // 4 == qc:   # diagonal k tile
                                            off = P * (kt % 4)
                                            nc.vector.tensor_mul(
                                                ptb[:, ki * SC + off:
                                                    ki * SC + off + P],
                                                ptb[:, ki * SC + off:
                                                    ki * SC + off + P],
                                                triT[:])
                                    for ki in range(2):
                                        kt = kt0 + ki
                                        avc0 = (P * (kt % 4)
                                                if kt // 4 == qc else 0)
                                        mm(
                                            av[64 * l2:64 * (l2 + 1),
                                               avc0:SC],
                                            v_sb[:, kt * JC + 64 * hl:
                                                 kt * JC + 64 * (hl + 1)],
                                            ptb[:, ki * SC + avc0:
                                                (ki + 1) * SC],
                                            start=(kt == 0),
                                            stop=(kt == kmax - 1),
                                            tile_position=(0, 64 * l2),
                                            skip_group_check=True)
                            nc.vector.tensor_copy(
                                ctx_sb[jt][:, qc * SC:(qc + 1) * SC], av[:])

            # ---------- output projection ----------------------------------
            with tc.tile_pool(name="wppool", bufs=1) as wppool, \
                 tc.tile_pool(name="postg", bufs=3) as postg, \
                 tc.tile_pool(name="ps_po", bufs=2, space="PSUM") as ps_po:
                wp_sb = wppool.tile([P, 2 * D], f32, tag="wp")
                nc.sync.dma_start(wp_sb.rearrange("p (c e) -> p c e", c=2),
                                  wp.rearrange("(c p) e -> p c e", p=P))
                # per-head K=64 matmuls (row-packed pairs run concurrently);
                # the 1/l normalization rides the PSUM->SBUF combine as a
                # per-partition (= per query row) scalar.
                for st in range(NQT if "proj" in phases else 0):
                    postage = postg.tile([P, D], f32, tag="po")
                    for ec in range(2):
                        pp = [ps_po.tile([P, SC], f32, tag=f"pp{h}",
                                         name=f"pp{h}") for h in range(HPC)]
                        for jt in range(2):
                            for l2 in range(2):
                                hl = 2 * jt + l2
                                mm(
                                    pp[hl][:],
                                    ctx_sb[jt][64 * l2:64 * (l2 + 1),
                                               st * P:(st + 1) * P],
                                    wp_sb[64 * l2:64 * (l2 + 1),
                                          jt * D + ec * SC:
                                          jt * D + (ec + 1) * SC],
                                    start=True, stop=True,
                                    tile_position=(64 * l2, 0))
                        sg = postage[:, ec * SC:(ec + 1) * SC]
                        nc.vector.tensor_scalar_mul(sg, pp[0][:],
                                                    rst[0][:, st:st + 1])
                        for hl in range(1, HPC):
                            nc.vector.scalar_tensor_tensor(
                                sg, pp[hl][:], rst[hl][:, st:st + 1], sg,
                                op0=ALU.mult, op1=ALU.add)
                    nc.sync.dma_start(po[st * P:(st + 1) * P, :], postage[:])

    nc.compile()
    return nc


def _get_program():
    if "nc" not in _cache:
        import os
        _cache["nc"] = _build_program(
            mm_dtype=os.environ.get("KERNEL_MM_DTYPE", "float32"))
    return _cache["nc"]


def _make_in_maps(hidden_states, w_attn, b_attn):
    hs = np.ascontiguousarray(np.asarray(hidden_states, dtype=np.float32))
    wa = np.asarray(w_attn, dtype=np.float32)
    ba = np.asarray(b_attn, dtype=np.float32)
    in_maps = []
    for c in range(NCORES):
        b = c // 4
        j0 = (c % 4) * HPC * HD
        in_maps.append({
            "xT": np.ascontiguousarray(hs[b].T),
            "wq": np.ascontiguousarray(wa[:, j0:j0 + JC]),
            "wk": np.ascontiguousarray(wa[:, D + j0:D + j0 + JC]),
            "wv": np.ascontiguousarray(wa[:, 2 * D + j0:2 * D + j0 + JC]),
            "bq": np.ascontiguousarray(ba[j0:j0 + JC].reshape(JC, 1)),
            "bk": np.ascontiguousarray(ba[D + j0:D + j0 + JC].reshape(JC, 1)),
            "bv": np.ascontiguousarray(
                ba[2 * D + j0:2 * D + j0 + JC].reshape(1, JC)),
        })
    return in_maps


def _make_wp_maps(w_proj):
    wpf = np.asarray(w_proj, dtype=np.float32)
    return [np.ascontiguousarray(wpf[(c % 4) * JC:(c % 4) * JC + JC, :])
            for c in range(NCORES)]


def kernel(hidden_states, w_attn, b_attn, w_proj, b_proj, _trace=False):
    from concourse.bass_utils import run_bass_kernel_spmd

    nc = _get_program()
    in_maps = _make_in_maps(hidden_states, w_attn, b_attn)
    wp_maps = _make_wp_maps(w_proj)
    for c in range(NCORES):
        in_maps[c]["wp"] = wp_maps[c]

    kw = {}
    if _trace:
        kw = dict(trace=True)
    res = run_bass_kernel_spmd(nc, in_maps, core_ids=list(range(NCORES)), **kw)

    attn_w = np.empty((B, H, S, S), np.float32)
    out = np.zeros((B, S, D), np.float32)
    for c in range(NCORES):
        b = c // 4
        h0 = (c % 4) * HPC
        attn_w[b, h0:h0 + HPC] = res.results[c]["pw"]
        out[b] += res.results[c]["po"]
    out += np.asarray(b_proj, dtype=np.float32)[None, None, :]
    if _trace:
        return (out, attn_w), res
    return out, attn_w
